# revision 2
# baseline (speedup 1.0000x reference)
"""AttnBlock2D Trainium2 kernel, v2: fp8 DoubleRow + ACT-paced schedule.

Math (per batch element, one per core):
    q = Wq @ x_self + bq; k = Wk @ x_cross (bk dropped: softmax-invariant)
    v = Wv @ x_cross (bv folded into the residual on host)
    per head h: scores = q_h^T k_h / 8; softmax over keys; o_h = attn @ v_h
    y = Wout @ o + bout + x_self

Numerics: x and Wq/Wk/Wv/Wout are host-quantized fp8e4 (weights x16);
projections and attention-V matmuls run in DoubleRow mode (K=256/instr,
2x bf16 rate).  q,k stay bf16 (scores matmuls bf16, head pairs overlap
at PE row bases 0/64).  E = exp(s/8 - 2) written fp8 by ACT directly
(the -2 offset cancels in softmax and keeps E < 240 = TRN fp8e4 max).

vt layout per head: [64 ones-columns | 64 v-columns(=16v)].  The U
DoubleRow matmul then emits the softmax denominator REPLICATED on psum
rows 0:64 and 16*V^T E on rows 64:128, so normalize is just:
    R[0:64] = reciprocal_approx_fast(psum[0:64])   (psum-in, base0->base0)
    o       = psum[64:128] * R                     (TT: in0 may be base 64)
(custom-DVE ops require base-partition-0 in/out; TT requires out/in1 at
base 0 — verified on HW.)  o = 16*O stored fp8; Y = (16Wout)^T(16O) =
256*y_attn; final STT does ps/256 + resid' with resid' = x_self + bout
+ Wout@bv folded on host.

Schedule: ACT (64 exps x [128,1024] from psum, ~1.12us each) is the
critical engine.  PSUM banks: sc0/sc1 [128,1024] scores ping-pong by
n-half (reused for Y in the epilogue), ub0/ub1 U accumulators n-half 0
(in-round, lagged 2 t-steps), pj0/pj1 projections (round 0) then U
accumulators n-half 1.

DMA: inputs live in two combined DRAM tensors (xall = xc|xs, wall =
wk|wq|wv|wo) so each dma_start moves 2-4KB contiguous per partition
(per-queue DMA rate is packet-size-bound).  Triggers cost ~600ns on the
issuing engine's queue, so they are split between the Sync and ACT HWDGE
rings; resid'/wo ride the gpsimd SWDGE queue behind a gate-copy so the
prologue-critical 1.75MB gets the full ring first.
"""

import os
from contextlib import ExitStack

import ml_dtypes
import numpy as np

import concourse.bass as bass
import concourse.tile as tile
from concourse import bacc, mybir

B = 8
C = 512
HEADS = 8
HD = 64
N = 1024
N_CORES = 8
HDP = 2 * HD  # per-head vt block: 64 ones + 64 v cols

F32 = mybir.dt.float32
BF16 = mybir.dt.bfloat16
FP8 = mybir.dt.float8e4
DR = mybir.MatmulPerfMode.DoubleRow
NP_FP8 = ml_dtypes.float8_e4m3

WSCALE = 16.0       # host scale on Wq/Wk/Wv/Wout before fp8 quantization
EXP_OFF = -2.0      # exp(s/8 - 2): softmax-invariant, keeps E <= ~110 < 240
EXP_SCALE = 1.0 / (8.0 * WSCALE * WSCALE)  # psum scores carry 256x
YSCALE = 1.0 / (WSCALE * WSCALE)           # y psum carries 256x

# col offsets in the combined weight tensor
WOFF = {"wk": 0, "wq": 2048, "wv": 4096, "wo": 6144}


def build():
    nc = bacc.Bacc("TRN2", target_bir_lowering=False, debug=False,
                   num_devices=N_CORES)

    # Combined DRAM tensors (host-prearranged, see make_in_maps):
    #  xall: [128, 8192] = xc | xs, each [128,4096] with
    #        col = tp*2048 + j*1024 + n          (channel c = tp*256+j*128+p)
    #  wall: [128, 8192] = wk | wq | wv | wo, each [128,2048]:
    #        wk/wq/wo col = tp*1024 + mt*256 + j*128 + m
    #        wv col = tp*1024 + j*512 + m
    #  rall: [128, 4096] = resid' with col = mt*1024 + n (row c = mt*128+p)
    xall_d = nc.dram_tensor("xall", [128, 8192], FP8, kind="ExternalInput").ap()
    wall_d = nc.dram_tensor("wall", [128, 8192], FP8, kind="ExternalInput").ap()
    bq_d = nc.dram_tensor("bq16", [128, 4], F32, kind="ExternalInput").ap()
    rall_d = nc.dram_tensor("rall", [128, 4096], F32, kind="ExternalInput").ap()
    y_d = nc.dram_tensor("y", [C, N], F32, kind="ExternalOutput").ap()

    MUL = mybir.AluOpType.mult
    ADD = mybir.AluOpType.add
    EXP = mybir.ActivationFunctionType.Exp

    with tile.TileContext(nc) as tc, ExitStack() as ctx:
        persist = ctx.enter_context(tc.tile_pool(name="persist", bufs=1))
        ppool = ctx.enter_context(tc.tile_pool(name="psum", bufs=1,
                                               space="PSUM"))
        epool = ctx.enter_context(tc.tile_pool(name="epool", bufs=2))
        npool = ctx.enter_context(tc.tile_pool(name="norm", bufs=1))
        ypool = ctx.enter_context(tc.tile_pool(name="yout", bufs=2))

        # ---- persistent SBUF tiles -------------------------------------
        bq_s = persist.tile([128, 4], F32, tag="bq", name="bq")
        nb_s = persist.tile([128, 1], F32, tag="nb", name="nb")
        wall_s = persist.tile([128, 8192], FP8, tag="wall", name="wall")
        xall_s = persist.tile([128, 8192], FP8, tag="xall", name="xall")
        q_s = [persist.tile([128, N], BF16, tag=f"q{i}", name=f"q{i}")
               for i in range(4)]
        k_s = [persist.tile([128, N], BF16, tag=f"k{i}", name=f"k{i}")
               for i in range(4)]
        # vt pair tiles: [p, (j, h, d128)]; d 0:64 = ones, 64:128 = 16*v
        vt_s = [persist.tile([128, 2 * HEADS * HDP], FP8, tag=f"vt{i}",
                             name=f"vt{i}") for i in range(4)]
        # o pair tiles: [p, (jj, n)] = 16*O rows (2dp+jj)*128+p
        o_s = [persist.tile([128, 2048], FP8, tag=f"o{i}", name=f"o{i}")
               for i in range(2)]
        rall_s = persist.tile([128, 4096], F32, tag="rall", name="rall")

        # constants first: nb via gpsimd (no DMA dep), ones cols, exp warm
        nc.gpsimd.memset(nb_s[:], EXP_OFF)
        warm = npool.tile([1, 8], F32, tag="warm", name="warm")
        nc.scalar.activation(warm[:, 0:4], nb_s[0:1, 0:1].to_broadcast(
            (1, 4)), EXP)
        for i in range(4):
            ones = vt_s[i][:].rearrange("p (j h d) -> p j h d", j=2, d=HDP)[
                :, :, :, 0:HD]
            nc.gpsimd.memset(ones, 1.0)

        # ---- DMA loads: prologue-critical transfers split across the two
        # HWDGE rings (sync + scalar); each trigger costs ~600ns of its
        # engine's queue.
        # critical loads split across the two HWDGE rings; whole-slice
        # transfers keep the per-partition runs at 4KB (ring rate is
        # packet-size-bound)
        nc.sync.dma_start(wall_s[:, 0:4096], wall_d[:, 0:4096])        # wk|wq
        nc.scalar.dma_start(xall_s[:, 0:4096], xall_d[:, 0:4096])      # xc
        nc.scalar.dma_start(xall_s[:, 4096:8192], xall_d[:, 4096:8192])  # xs
        nc.sync.dma_start(wall_s[:, 4096:6144], wall_d[:, 4096:6144])  # wv
        nc.sync.dma_start(bq_s[:], bq_d)                               # bq
        # wo and resid' ride the gpsimd SWDGE queue, gated (see main loop)

        # PE warm-up: tiny dummy matmuls while the input DMA drains, so the
        # HAM clock gate is at 8/8 (2.4GHz) when the first projections run
        wdum = ppool.tile([128, N], F32, tag="sc0", name="wdum")
        for r in range(16):
            nc.tensor.matmul(wdum[0:1, 0:1], lhsT=nb_s[:], rhs=nb_s[:],
                             start=True, stop=True)

        # ---- AP helpers ------------------------------------------------
        def w_lhsT(name, tp, mt):
            off = WOFF[name] + tp * 1024
            return wall_s[:, off:off + 1024].rearrange(
                "p (mt j m) -> p mt j m", mt=4, j=2)[:, mt]

        def x_rhs(which, tp, nh):
            off = (0 if which == "xc" else 4096) + tp * 2048
            return xall_s[:, off:off + 2048].rearrange(
                "p (j n) -> p j n", j=2)[:, :, nh * 512:(nh + 1) * 512]

        def qk_proj(which, mt, nh, ptag):
            wname, xname = ("wq", "xs") if which == "q" else ("wk", "xc")
            dst = (q_s if which == "q" else k_s)[mt][
                :, nh * 512:(nh + 1) * 512]
            ps = ppool.tile([128, 512], F32, tag=ptag, name=f"p{which}")
            for tp in range(2):
                nc.tensor.matmul(ps[:], lhsT=w_lhsT(wname, tp, mt),
                                 rhs=x_rhs(xname, tp, nh),
                                 start=(tp == 0), stop=(tp == 1),
                                 perf_mode=DR)
            if which == "q":
                nc.vector.tensor_scalar_add(dst, ps[:], bq_s[:, mt:mt + 1])
            else:
                nc.vector.tensor_copy(out=dst, in_=ps[:])

        def v_proj(t, ptag):
            ps = ppool.tile([128, 512], F32, tag=ptag, name="pv")
            for tp in range(2):
                lhsT = xall_s[:, tp * 2048:(tp + 1) * 2048].rearrange(
                    "p (j n) -> p j n", j=2)[:, :, t * 128:(t + 1) * 128]
                rhs = wall_s[:, 4096 + tp * 1024: 4096 + (tp + 1) * 1024
                             ].rearrange("p (j m) -> p j m", j=2)
                nc.tensor.matmul(ps[:], lhsT=lhsT, rhs=rhs,
                                 start=(tp == 0), stop=(tp == 1),
                                 perf_mode=DR)
            vdst = vt_s[t // 2][:].rearrange("p (j h d) -> p j h d", j=2,
                                             d=HDP)[:, t % 2, :, HD:HDP]
            vsrc = ps[:].rearrange("p (h d) -> p h d", d=HD)
            nc.vector.tensor_copy(out=vdst, in_=vsrc)

        def scores(p, t, nh):
            ps = ppool.tile([128, N], F32, tag=f"sc{nh}", name="sc")
            for i, base in enumerate((0, 64)):
                nc.tensor.matmul(
                    ps[:, i * 512:(i + 1) * 512],
                    lhsT=k_s[p][base:base + 64, t * 128:(t + 1) * 128],
                    rhs=q_s[p][base:base + 64, nh * 512:(nh + 1) * 512],
                    start=True, stop=True)
            return ps

        def expu(e_t, p, t, nh, ps):
            off = (t // 2) * 4096 + (t % 2) * 2048 + nh * 1024
            nc.scalar.activation(e_t[:, off:off + 1024], ps[:], EXP,
                                 scale=EXP_SCALE, bias=nb_s[:])

        def e_rhs(e_t, tp, nh, i):
            r = e_t[:].rearrange("p (tp j c) -> p tp j c", tp=4, j=2)
            a = nh * 1024 + i * 512
            return r[:, tp, :, a:a + 512]

        def u_mm(ups, e_t, p, i, nh, tp):
            h = 2 * p + i
            lhsT = vt_s[tp][:].rearrange("p (j h d) -> p j h d", j=2,
                                         d=HDP)[:, :, h, :]
            nc.tensor.matmul(ups[(i, nh)][:], lhsT=lhsT,
                             rhs=e_rhs(e_t, tp, nh, i),
                             start=(tp == 0), stop=(tp == 3), perf_mode=DR)

        def u_open(ups, i, nh, ptag):
            if (i, nh) not in ups:
                ups[(i, nh)] = ppool.tile([128, 512], F32, tag=ptag,
                                          name=f"u{i}{nh}")

        def normalize(ups, p, i, nh):
            """o rows = psum[64:128] * recip(psum[0:64]) -> fp8 o tile."""
            dp, jj = p // 2, p % 2
            cols = slice(jj * 1024 + nh * 512, jj * 1024 + nh * 512 + 512)
            up = ups[(i, nh)]
            R = npool.tile([64, 512], F32, tag=f"R{i}{nh}", name="R")
            nc.vector.reciprocal_approx_fast(R[:], up[0:64, :])
            if i == 0:
                nc.vector.tensor_tensor(o_s[dp][0:64, cols], up[64:128, :],
                                        R[:], op=MUL)
            else:
                stg = npool.tile([64, 512], FP8, tag=f"stg{nh}", name="stg")
                nc.vector.tensor_tensor(stg[:], up[64:128, :], R[:], op=MUL)
                nc.vector.tensor_copy(out=o_s[dp][64:128, cols], in_=stg[:])

        # ---- prologue: pair-0 Q/K (both n-halves), K first --------------
        qk_proj("k", 0, 0, "pj0")
        qk_proj("q", 0, 0, "pj1")
        qk_proj("k", 0, 1, "pj0")
        qk_proj("q", 0, 1, "pj1")

        # V(7) must be emitted by t=6: the t=7 n-half-0 tail reads vt_s[3]
        inj0 = {
            0: [("k", 1, 0), ("k", 1, 1)],
            1: [("q", 1, 0), ("q", 1, 1), ("v", 0)],
            2: [("k", 2, 0), ("k", 2, 1), ("v", 1)],
            3: [("q", 2, 0), ("q", 2, 1), ("v", 2)],
            4: [("k", 3, 0), ("k", 3, 1), ("v", 3)],
            5: [("q", 3, 0), ("q", 3, 1), ("v", 4)],
            6: [("v", 5), ("v", 6), ("v", 7)],
            7: [],
        }

        e_tiles = {}
        ups_all = {}

        def tail_late(pm):
            """Round pm's trailing late-n-half U accumulation + normalize,
            emitted after the next round's first scores/exps (epilogue for
            pm=3, whose rounds run n-half-1 first)."""
            ups, e_t = ups_all[pm], e_tiles[pm]
            if pm >= 1:
                nhB = 0 if pm == 3 else 1
                for i in range(2):
                    u_mm(ups, e_t, pm, i, nhB, 3)
            if 1 <= pm < 3:
                for i in range(2):
                    normalize(ups, pm, i, 1)

        for p in range(4):
            e_t = epool.tile([128, 16384], FP8, tag="e", name=f"e{p}")
            e_tiles[p] = e_t
            ups = {}
            ups_all[p] = ups
            pj_alt = 0
            # round 3 runs n-half 1 first so its (heavier) normalize chain
            # hides under the final exp and Y can start on n-half 1 at once
            nhA, nhB = (1, 0) if p == 3 else (0, 1)
            tagA = "pj" if nhA else "ub"
            for t in range(8):
                psA = scores(p, t, nhA)
                expu(e_t, p, t, nhA, psA)
                psB = scores(p, t, nhB)
                if t == 7:
                    # early-n-half tail: runs on PE/DVE during the last exp
                    for i in range(2):
                        u_open(ups, i, nhA, f"{tagA}{i}")
                        u_mm(ups, e_t, p, i, nhA, 3)
                    for i in range(2):
                        normalize(ups, p, i, nhA)
                expu(e_t, p, t, nhB, psB)
                if t == 0 and p >= 1:
                    tail_late(p - 1)
                if p == 0:
                    for item in inj0[t]:
                        tag = f"pj{pj_alt}"
                        pj_alt ^= 1
                        if item[0] == "v":
                            v_proj(item[1], tag)
                        else:
                            qk_proj(item[0], item[1], item[2], tag)
                if p == 1 and t == 0:
                    # rs/wo on the gpsimd SWDGE queue, gated behind a copy
                    # that reads this round's first exp output
                    nc.gpsimd.tensor_copy(out=rall_s[0:1, 0:1],
                                          in_=e_t[0:1, 0:1])
                    nc.gpsimd.dma_start(rall_s[:], rall_d)
                    nc.gpsimd.dma_start(wall_s[:, 6144:8192],
                                        wall_d[:, 6144:8192])
                if p == 1 and t in (0, 1):
                    # pair-0 n-half-1 U, deferred (pj banks were projections)
                    pm = 0
                    for tp in (0, 1) if t == 0 else (2, 3):
                        for i in range(2):
                            u_open(ups_all[pm], i, 1, f"pj{i}")
                            u_mm(ups_all[pm], e_tiles[pm], pm, i, 1, tp)
                    if t == 1:
                        for i in range(2):
                            normalize(ups_all[pm], pm, i, 1)
                # in-round lagged U: e-pair tp complete after t = 2tp+1
                if t in (3, 4, 6):
                    tp = {3: 0, 4: 1, 6: 2}[t]
                    for i in range(2):
                        u_open(ups, i, 0, f"ub{i}")
                        u_mm(ups, e_t, p, i, 0, tp)
                    if p >= 1:
                        for i in range(2):
                            u_open(ups, i, 1, f"pj{i}")
                            u_mm(ups, e_t, p, i, 1, tp)

        # ---- epilogue --------------------------------------------------
        # round 3 ran n-half 1 first (normalized under the last exp), so Y
        # n-half 1 starts immediately; pair-3 n-half-0 normalize overlaps it.
        p = 3
        ups = ups_all[p]
        tail_late(3)  # pair-3 n-half-0 e-pair-3 accumulation

        def y_mms(mt, nh, psap):
            for dp in range(2):
                rhs = o_s[dp][:].rearrange("p (j n) -> p j n", j=2)[
                    :, :, nh * 512:(nh + 1) * 512]
                nc.tensor.matmul(psap, lhsT=w_lhsT("wo", dp, mt), rhs=rhs,
                                 start=(dp == 0), stop=(dp == 1),
                                 perf_mode=DR)

        y_tiles = {mt: ypool.tile([128, N], F32, tag=f"y{mt % 2}", name="y_t")
                   for mt in range(4)}
        ps_full = {mt: ppool.tile([128, N], F32, tag=f"sc{mt}", name="yps")
                   for mt in range(2)}
        ps_half = {}
        for mt in range(2):
            ps_half[(mt, 0)] = ps_full[mt][:, 0:512]
            ps_half[(mt, 1)] = ps_full[mt][:, 512:1024]
        for (mt, nh), tag in (((2, 1), "pj0"), ((3, 1), "pj1"),
                              ((2, 0), "ub0"), ((3, 0), "ub1")):
            ps_half[(mt, nh)] = ppool.tile([128, 512], F32, tag=tag,
                                           name="yph")[:]

        for mt in range(4):
            y_mms(mt, 1, ps_half[(mt, 1)])
        for i in range(2):
            normalize(ups, p, i, 0)
        for mt in range(4):
            nc.vector.scalar_tensor_tensor(
                y_tiles[mt][:, 512:1024], ps_half[(mt, 1)], YSCALE,
                rall_s[:, mt * 1024 + 512:(mt + 1) * 1024], op0=MUL, op1=ADD)
            # y half-transfers alternate rings so the 2MB drains in parallel
            eng = nc.scalar if mt % 2 else nc.sync
            eng.dma_start(y_d[mt * 128:(mt + 1) * 128, 512:1024],
                          y_tiles[mt][:, 512:1024])
        for mt in range(4):
            y_mms(mt, 0, ps_half[(mt, 0)])
        for mt in range(4):
            nc.vector.scalar_tensor_tensor(
                y_tiles[mt][:, 0:512], ps_half[(mt, 0)], YSCALE,
                rall_s[:, mt * 1024:mt * 1024 + 512], op0=MUL, op1=ADD)
            eng = nc.sync if mt % 2 else nc.scalar
            eng.dma_start(y_d[mt * 128:(mt + 1) * 128, 0:512],
                          y_tiles[mt][:, 0:512])

    nc.compile()
    return nc


def make_in_maps(self_feature, cross_feature, Wq, bq, Wk, bk, Wv, bv, Wout,
                 bout):
    f32 = np.float32
    sf = np.asarray(self_feature, f32).reshape(B, C, N)
    cf = np.asarray(cross_feature, f32).reshape(B, C, N)
    Wq = np.asarray(Wq, f32)
    Wk = np.asarray(Wk, f32)
    Wv = np.asarray(Wv, f32)
    Wout = np.asarray(Wout, f32)
    bq16 = np.ascontiguousarray(
        (WSCALE * np.asarray(bq, f32)).reshape(4, 128).T).astype(f32)
    bout2 = (np.asarray(bout, f32) + Wout @ np.asarray(bv, f32)).astype(f32)
    del bk  # softmax-invariant

    def q8(a):
        return np.clip(a, -240.0, 240.0).astype(NP_FP8)

    def x_dr(x):
        return np.ascontiguousarray(
            x.reshape(2, 2, 128, N).transpose(2, 0, 1, 3).reshape(128, 4096))

    def w_mtj(wT):
        a = wT.reshape(2, 2, 128, 4, 128)          # tp j p mt m
        return np.ascontiguousarray(
            a.transpose(2, 0, 3, 1, 4).reshape(128, 2048))

    def w_jm(wT):
        a = wT.reshape(2, 2, 128, 512)             # tp j p m
        return np.ascontiguousarray(
            a.transpose(2, 0, 1, 3).reshape(128, 2048))

    wall = np.concatenate([w_mtj(WSCALE * Wk.T), w_mtj(WSCALE * Wq.T),
                           w_jm(WSCALE * Wv.T), w_mtj(WSCALE * Wout.T)],
                          axis=1)
    wall8 = q8(wall)
    in_maps = []
    for b in range(B):
        resid = sf[b] + bout2[:, None]
        rall = np.ascontiguousarray(
            resid.reshape(4, 128, N).transpose(1, 0, 2).reshape(128, 4096))
        xall = np.concatenate([x_dr(cf[b]), x_dr(sf[b])], axis=1)
        in_maps.append({
            "xall": q8(xall),
            "wall": wall8,
            "bq16": bq16,
            "rall": rall,
        })
    return in_maps


_NC = None


def kernel(self_feature, cross_feature, Wq, bq, Wk, bk, Wv, bv, Wout, bout):
    from concourse.bass_utils import run_bass_kernel_spmd

    global _NC
    if _NC is None:
        _NC = build()
    in_maps = make_in_maps(self_feature, cross_feature, Wq, bq, Wk, bk, Wv,
                           bv, Wout, bout)
    res = run_bass_kernel_spmd(_NC, in_maps, core_ids=list(range(N_CORES)))
    y = np.stack([res.results[b]["y"].reshape(C, 32, 32) for b in range(B)])
    return np.ascontiguousarray(y.astype(np.float32))
